# revision 18
# baseline (speedup 1.0000x reference)
"""MultiHeadAttention (B=2, S=2048, D=1024, H=16) on 8 Trainium2 NeuronCores.

Sharding: core c -> batch b = c // 4, head group g = c % 4 (4 of 16 heads =
256 of the 1024 projection columns). Cores are fully independent (no
collectives): each core writes its partial output projection (rows x D for
its 256 ctx columns) as fp16, and the host sums the 4 partials per batch
and adds bo. Per-core timeline:

  P1-P2: q/k projections for the core's 4 heads over the full sequence,
         produced directly transposed/head-major: qT,kT [256, S] fp16.
  P3:    v projection in natural layout [S, 4*65] fp16, with a ones column
         appended per head (yields softmax denominators for free in P4).
  P4:    per head and 1024-wide query slice: scoresT[k,q] = kT_blk.T @ qT
         (fp16 operands, fp32 PSUM) for two key blocks into one 2-bank
         PSUM tile, one 1024-wide exp on ScalarE with the 1/sqrt(64) scale
         folded in (scores are O(10): no max pass needed), then
         oT'[65,q] += v_blk.T @ pT. Row 64 of oT' is the softmax sum;
         normalize via DVE reciprocal + Pool partition_broadcast (keeps the
         normalize entirely off the Tensor engine) -> ctxT [256, S] fp16.
  P5:    partial output projection; each 128x512 tile is emitted deferred,
         interleaved into the next query slice's attention loop so the PE
         bubbles left by the exp-paced softmax get filled with matmul work.

Matmul operands are fp16 (1 cycle/row on the PE); accumulation is fp32.
fp8/DoubleRow was evaluated and rejected: quantizing any attention operand
to fp8 puts 4-9% noise directly on the output (softmax averages signal and
noise alike), far over the 2e-2 budget.
"""

import numpy as np

import concourse.bacc as bacc
import concourse.mybir as mybir
from concourse.tile import TileContext
from concourse.bass_utils import run_bass_kernel_spmd

F32 = mybir.dt.float32
F16 = mybir.dt.float16

B, S, D = 2, 2048, 1024
H, DH = 16, 64
NCORES = 8
HPG = 4            # heads per core
DG = HPG * DH      # 256 projection cols per core
IC = D // 128      # 8 contraction chunks for the projections
KC = S // 128      # 16 key blocks
VW = DH + 1        # 65 = head dim + ones column

_NC_CACHE = {}


def _build_nc():
    nc = bacc.Bacc("TRN2", target_bir_lowering=False, num_devices=NCORES)

    xq = nc.dram_tensor("xq", [IC, 128, S], F16, kind="ExternalInput")
    xk = nc.dram_tensor("xk", [IC, 128, S], F16, kind="ExternalInput")
    xv = nc.dram_tensor("xv", [IC, 128, S], F16, kind="ExternalInput")
    wq = nc.dram_tensor("wq", [IC, 128, DG], F16, kind="ExternalInput")
    wk = nc.dram_tensor("wk", [IC, 128, DG], F16, kind="ExternalInput")
    wv = nc.dram_tensor("wv", [IC, 128, DG], F16, kind="ExternalInput")
    wo = nc.dram_tensor("wo", [2, 128, D], F16, kind="ExternalInput")
    bq2 = nc.dram_tensor("bq2", [2, 128], F32, kind="ExternalInput")
    bk2 = nc.dram_tensor("bk2", [2, 128], F32, kind="ExternalInput")
    bvb = nc.dram_tensor("bvb", [128, DG], F32, kind="ExternalInput")
    out = nc.dram_tensor("out", [S, D], F16, kind="ExternalOutput")

    Exp = mybir.ActivationFunctionType.Exp

    with nc.allow_low_precision(reason="fp16 attention internals"), \
            TileContext(nc) as tc:
        with (
            tc.tile_pool(name="persist", bufs=1) as pers,
            tc.tile_pool(name="pt", bufs=3) as ptp,
            tc.tile_pool(name="small", bufs=4) as small,
            tc.tile_pool(name="outp", bufs=3) as outp,
            tc.tile_pool(name="ps", bufs=2, space="PSUM") as ps,
            tc.tile_pool(name="ps2", bufs=2, space="PSUM") as ps2,
            tc.tile_pool(name="psov", bufs=2, space="PSUM") as psov,
        ):
            # ---- persistent SBUF ----
            wq_sb = pers.tile([128, IC * DG], F16, tag="wq")
            wk_sb = pers.tile([128, IC * DG], F16, tag="wk")
            wv_sb = pers.tile([128, IC * DG], F16, tag="wv")
            wo_sb = pers.tile([128, 2 * D], F16, tag="wo")
            qt_sb = [pers.tile([128, S], F16, tag=f"qt{i}", name=f"qt{i}")
                     for i in range(2)]
            ktz_sb = [pers.tile([128, 2 * S], F16, tag=f"kt{i}", name=f"ktz{i}")
                      for i in range(2)]
            v_sb = pers.tile([128, KC * HPG * VW], F16, tag="v")
            ctxT_sb = [pers.tile([128, S], F16, tag=f"cx{i}", name=f"cx{i}")
                       for i in range(2)]
            bq_sb = pers.tile([128, 2], F32, tag="bq")
            bk_sb = pers.tile([128, 2], F32, tag="bk")
            bvb_sb = pers.tile([128, DG], F32, tag="bvb")
            recip_sb = pers.tile([1, 512], F16, tag="recip")
            xk_sb = pers.tile([128, IC * S], F16, tag="xk")
            xv_sb = pers.tile([128, IC * S], F16, tag="xv")
            xq_sb = pers.tile([128, IC * S], F16, tag="xq")

            def load_w(eng, wsb, wdr, kchunks):
                eng.dma_start(
                    out=wsb.rearrange("p (k n) -> p k n", k=kchunks),
                    in_=wdr.rearrange("k p n -> p k n"),
                )

            # xk + wk first (first matmul depends only on these); the
            # rest of the inputs stream in behind them.
            def load_x(eng, xsb, xdr):
                # slice-granular loads so consumers unblock incrementally
                for s4 in range(4):
                    w = slice(s4 * 512, (s4 + 1) * 512)
                    eng.dma_start(
                        out=xsb.rearrange("p (k n) -> p k n", k=IC)[:, :, w],
                        in_=xdr[:, :, w].rearrange("k p n -> p k n"),
                    )

            load_w(nc.sync, wk_sb, wk, IC)
            load_x(nc.sync, xk_sb, xk)
            nc.sync.dma_start(out=bk_sb[:], in_=bk2.rearrange("c p -> p c"))
            load_x(nc.scalar, xv_sb, xv)
            load_w(nc.scalar, wv_sb, wv, IC)
            nc.scalar.dma_start(out=bvb_sb[:], in_=bvb[:])
            load_x(nc.scalar, xq_sb, xq)
            load_w(nc.scalar, wq_sb, wq, IC)
            nc.scalar.dma_start(out=bq_sb[:], in_=bq2.rearrange("c p -> p c"))
            load_w(nc.scalar, wo_sb, wo, 2)
            for z in range(2):
                nc.vector.memset(ktz_sb[z][:], 0.0)
            nc.vector.memset(
                v_sb.rearrange("p (k h e) -> p k h e",
                               h=HPG, e=VW)[:, :, :, DH], 1.0)

            # ---- deferred-emission queue (fills PE bubbles) ----
            deferred = []

            def flush(n=1 << 30):
                cnt = min(n, len(deferred))
                for fn in deferred[:cnt]:
                    fn()
                del deferred[:cnt]

            # ---- helper: project one 512-wide slice of x ----
            def qk_proj_slice(xsb, wsb3, sink, s4):
                xt3 = xsb.rearrange("p (k n) -> p k n", k=IC)[
                    :, :, s4 * 512:(s4 + 1) * 512]
                for oc in range(2):
                    acc = ps.tile([128, 512], F32, tag="mm", name=f"acc_{s4}")
                    for ic in range(IC):
                        nc.tensor.matmul(
                            acc[:],
                            wsb3[:, ic, oc * 128:(oc + 1) * 128],
                            xt3[:, ic, :],
                            start=(ic == 0),
                            stop=(ic == IC - 1),
                        )
                    sink(oc, s4, acc)

            def qt_sink(oc, s4, acc):
                nc.vector.tensor_scalar_add(
                    qt_sb[oc][:, s4 * 512:(s4 + 1) * 512], acc[:],
                    bq_sb[:, oc:oc + 1],
                )

            def ktz_sink(oc, s4, acc):
                # head hsel of this pair -> col block (2*kb + hsel)*128, with
                # only d-rows 64*hsel..+64 populated (rest stays zero).
                z4 = ktz_sb[oc].rearrange("p (kb two m) -> p kb two m", two=2, m=128)
                a3 = acc.rearrange("p (kb m) -> p kb m", m=128)
                kb0 = 4 * s4
                for hsel in range(2):
                    nc.vector.tensor_scalar_add(
                        z4[64 * hsel:64 * hsel + 64, kb0:kb0 + 4, hsel, :],
                        a3[64 * hsel:64 * hsel + 64, :, :],
                        bk_sb[64 * hsel:64 * hsel + 64, oc:oc + 1],
                    )

            # ---- P1: k projection (q is projected inside the qs loop) ----
            wk3 = wk_sb.rearrange("p (k n) -> p k n", k=IC)
            wq3 = wq_sb.rearrange("p (k n) -> p k n", k=IC)
            for s4 in range(4):
                qk_proj_slice(xk_sb, wk3, ktz_sink, s4)

            # ---- P3: v projection -> [S, 4*65] fp16 with ones columns ----
            wv3 = wv_sb.rearrange("p (k n) -> p k n", k=IC)
            v4 = v_sb.rearrange("p (k h e) -> p k h e", h=HPG, e=VW)
            for s4 in range(4):
                xt3 = xv_sb.rearrange("p (k n) -> p k n", k=IC)[
                    :, :, s4 * 512:(s4 + 1) * 512]
                for j in range(4):  # key chunk kc = 4*s4 + j
                    kc = 4 * s4 + j
                    acc = ps.tile([128, 512], F32, tag="mm")
                    for ic in range(IC):
                        nc.tensor.matmul(
                            acc[:, 0:DG],
                            xt3[:, ic, j * 128:(j + 1) * 128],
                            wv3[:, ic, :],
                            start=(ic == 0),
                            stop=(ic == IC - 1),
                        )
                    nc.vector.tensor_add(
                        out=v4[:, kc, :, 0:DH],
                        in0=acc[:, 0:DG].rearrange("p (h e) -> p h e", e=DH),
                        in1=bvb_sb.rearrange("p (h e) -> p h e", e=DH),
                    )

            # ---- P4 + P5 per 512-wide query slice ----
            wo3 = wo_sb.rearrange("p (k n) -> p k n", k=2)

            def p5_tile(ib, oh):
                acc = ps.tile([128, 512], F32, tag="mm")
                for cc in range(2):
                    nc.tensor.matmul(
                        acc[:],
                        ctxT_sb[cc][:, ib * 128:(ib + 1) * 128],
                        wo3[:, cc, oh * 512:(oh + 1) * 512],
                        start=(cc == 0),
                        stop=(cc == 1),
                    )
                ot = outp.tile([128, 512], F16, tag="ot")
                nc.scalar.copy(out=ot[:], in_=acc[:])
                nc.sync.dma_start(
                    out=out[ib * 128:(ib + 1) * 128, oh * 512:(oh + 1) * 512],
                    in_=ot[:],
                )

            for qs in range(4):  # 512-wide query slice
                qlo = qs * 512
                if qs == 0:
                    qk_proj_slice(xq_sb, wq3, qt_sink, 0)
                for hg in range(HPG):
                    if hg == 3 and qs < 3:
                        # next slice's q projection: its DVE sink lands ahead
                        # of this head's normalize, so the next slice's first
                        # scores are never blocked on the Vector queue.
                        qk_proj_slice(xq_sb, wq3, qt_sink, qs + 1)
                    oc, ofs = hg // 2, 64 * (hg % 2)
                    hsel = hg % 2
                    ov = psov.tile([VW, 512], F32, tag="ov")
                    for kp in range(KC // 2):  # pairs of key blocks
                        sc = ps2.tile([128, 1024], F32, tag="sc")
                        for i in range(2):
                            kb = 2 * kp + i
                            nc.tensor.matmul(
                                sc[:, i * 512:(i + 1) * 512],
                                ktz_sb[oc][:, (2 * kb + hsel) * 128:
                                           (2 * kb + hsel + 1) * 128],
                                qt_sb[oc][:, qlo:qlo + 512],
                                start=True,
                                stop=True,
                            )
                        pt = ptp.tile([128, 1024], F16, tag="pt")
                        nc.scalar.activation(pt[:], sc[:], Exp, scale=0.125)
                        for i in range(2):
                            kb = 2 * kp + i
                            nc.tensor.matmul(
                                ov[:],
                                v_sb[:, (kb * HPG + hg) * VW:
                                     (kb * HPG + hg + 1) * VW],
                                pt[:, i * 512:(i + 1) * 512],
                                start=(kb == 0),
                                stop=(kb == KC - 1),
                            )
                        if kp in (3, 5, 7):
                            flush(1)  # a deferred P5 tile fills the PE bubble
                    # normalize: entirely off the Tensor engine
                    nc.vector.reciprocal(recip_sb[0:1, :], ov[DH:VW, :])
                    bcs = small.tile([DH, 512], F16, tag="bcs")
                    nc.gpsimd.partition_broadcast(bcs[:], recip_sb[0:1, :])
                    nc.vector.tensor_mul(
                        out=ctxT_sb[oc][ofs:ofs + DH, qlo:qlo + 512],
                        in0=ov[0:DH, :],
                        in1=bcs[:],
                    )

                # P5: partial output projection for the finished 512 rows,
                # deferred into the next slice's attention loop.
                for ibl in range(4):
                    ib = 4 * qs + ibl
                    for oh in range(2):
                        deferred.append(lambda ib=ib, oh=oh: p5_tile(ib, oh))
            flush()

    nc.compile()
    return nc


def _get_nc():
    if "nc" not in _NC_CACHE:
        _NC_CACHE["nc"] = _build_nc()
    return _NC_CACHE["nc"]


def _prep_inputs(Q, K, V, Wq, Wk, Wv, Wo, bq, bk, bv, bo):
    f = np.float32
    h = np.float16
    Q, K, V = (np.asarray(a, f) for a in (Q, K, V))
    Wq, Wk, Wv, Wo = (np.asarray(a, f) for a in (Wq, Wk, Wv, Wo))
    bq, bk, bv, bo = (np.asarray(a, f) for a in (bq, bk, bv, bo))

    xqs = [np.ascontiguousarray(Q[b].T).astype(h).reshape(IC, 128, S)
           for b in range(B)]
    xks = [np.ascontiguousarray(K[b].T).astype(h).reshape(IC, 128, S)
           for b in range(B)]
    xvs = [np.ascontiguousarray(V[b].T).astype(h).reshape(IC, 128, S)
           for b in range(B)]
    WqT, WkT, WvT, WoT = Wq.T, Wk.T, Wv.T, Wo.T

    in_maps = []
    for c in range(NCORES):
        b, g = c // 4, c % 4
        cols = slice(DG * g, DG * (g + 1))
        in_maps.append({
            "xq": xqs[b], "xk": xks[b], "xv": xvs[b],
            "wq": np.ascontiguousarray(WqT[:, cols], dtype=h).reshape(IC, 128, DG),
            "wk": np.ascontiguousarray(WkT[:, cols], dtype=h).reshape(IC, 128, DG),
            "wv": np.ascontiguousarray(WvT[:, cols], dtype=h).reshape(IC, 128, DG),
            "wo": np.ascontiguousarray(WoT[cols, :], dtype=h).reshape(2, 128, D),
            "bq2": np.ascontiguousarray(bq[cols]).reshape(2, 128),
            "bk2": np.ascontiguousarray(bk[cols]).reshape(2, 128),
            "bvb": np.ascontiguousarray(np.broadcast_to(bv[cols], (128, DG))),
        })
    return in_maps, bo


def _assemble(results, bo):
    out = np.empty((B, S, D), np.float32)
    for b in range(B):
        acc = results[4 * b]["out"].astype(np.float32)
        for g in range(1, 4):
            acc += results[4 * b + g]["out"].astype(np.float32)
        out[b] = acc + bo
    return out


def kernel(**inputs):
    nc = _get_nc()
    in_maps, bo = _prep_inputs(**inputs)
    res = run_bass_kernel_spmd(nc, in_maps, core_ids=list(range(NCORES)))
    return _assemble(res.results, bo)


# revision 19
# speedup vs baseline: 1.0736x; 1.0736x over previous
"""MultiHeadAttention (B=2, S=2048, D=1024, H=16) on 8 Trainium2 NeuronCores.

Sharding: core c -> batch b = c // 4, head group g = c % 4 (4 of 16 heads =
256 of the 1024 projection columns). Cores are fully independent (no
collectives): each core writes its partial output projection (rows x D for
its 256 ctx columns) as fp16, and the host sums the 4 partials per batch
and adds bo. Per-core timeline:

  P1-P2: q/k projections for the core's 4 heads over the full sequence,
         produced directly transposed/head-major: qT,kT [256, S] fp16.
  P3:    v projection in natural layout [S, 4*65] fp16, with a ones column
         appended per head (yields softmax denominators for free in P4).
  P4:    per head and 1024-wide query slice: scoresT[k,q] = kT_blk.T @ qT
         (fp16 operands, fp32 PSUM) for two key blocks into one 2-bank
         PSUM tile, one 1024-wide exp on ScalarE with the 1/sqrt(64) scale
         folded in (scores are O(10): no max pass needed), then
         oT'[65,q] += v_blk.T @ pT. Row 64 of oT' is the softmax sum;
         normalize via DVE reciprocal + Pool partition_broadcast (keeps the
         normalize entirely off the Tensor engine) -> ctxT [256, S] fp16.
  P5:    partial output projection; each 128x512 tile is emitted deferred,
         interleaved into the next query slice's attention loop (from key
         pair 3 on, so the previous slice's last normalize has drained).

The next slice's q projection is emitted before the current slice's last
head so its Vector-engine sink isn't queued behind the slow reciprocal.
Matmul operands are fp16 (1 cycle/row on the PE); accumulation is fp32.
fp8/DoubleRow was evaluated and rejected: quantizing any attention operand
to fp8 puts 4-9% noise directly on the output (softmax averages signal and
noise alike), far over the 2e-2 budget.
"""

import numpy as np

import concourse.bacc as bacc
import concourse.mybir as mybir
from concourse.tile import TileContext
from concourse.bass_utils import run_bass_kernel_spmd

F32 = mybir.dt.float32
F16 = mybir.dt.float16

B, S, D = 2, 2048, 1024
H, DH = 16, 64
NCORES = 8
HPG = 4            # heads per core
DG = HPG * DH      # 256 projection cols per core
IC = D // 128      # 8 contraction chunks for the projections
KC = S // 128      # 16 key blocks
VW = DH + 1        # 65 = head dim + ones column

_NC_CACHE = {}


def _build_nc():
    nc = bacc.Bacc("TRN2", target_bir_lowering=False, num_devices=NCORES)

    xq = nc.dram_tensor("xq", [IC, 128, S], F16, kind="ExternalInput")
    xk = nc.dram_tensor("xk", [IC, 128, S], F16, kind="ExternalInput")
    xv = nc.dram_tensor("xv", [IC, 128, S], F16, kind="ExternalInput")
    # weights host-packed per-partition-contiguous: one 4KB descriptor per
    # partition on load
    wq = nc.dram_tensor("wq", [128, IC * DG], F16, kind="ExternalInput")
    wk = nc.dram_tensor("wk", [128, IC * DG], F16, kind="ExternalInput")
    wv = nc.dram_tensor("wv", [128, IC * DG], F16, kind="ExternalInput")
    wo = nc.dram_tensor("wo", [128, 2 * D], F16, kind="ExternalInput")
    bq2 = nc.dram_tensor("bq2", [2, 128], F32, kind="ExternalInput")
    bk2 = nc.dram_tensor("bk2", [2, 128], F32, kind="ExternalInput")
    bvb = nc.dram_tensor("bvb", [128, DG], F32, kind="ExternalInput")
    out = nc.dram_tensor("out", [S, D], F16, kind="ExternalOutput")

    Exp = mybir.ActivationFunctionType.Exp

    with nc.allow_low_precision(reason="fp16 attention internals"), \
            TileContext(nc) as tc:
        with (
            tc.tile_pool(name="persist", bufs=1) as pers,
            tc.tile_pool(name="xin", bufs=4) as xin,
            tc.tile_pool(name="pt", bufs=3) as ptp,
            tc.tile_pool(name="small", bufs=4) as small,
            tc.tile_pool(name="outp", bufs=3) as outp,
            tc.tile_pool(name="ps", bufs=2, space="PSUM") as ps,
            tc.tile_pool(name="ps2", bufs=2, space="PSUM") as ps2,
            tc.tile_pool(name="psov", bufs=2, space="PSUM") as psov,
        ):
            # ---- persistent SBUF ----
            wq_sb = pers.tile([128, IC * DG], F16, tag="wq")
            wk_sb = pers.tile([128, IC * DG], F16, tag="wk")
            wv_sb = pers.tile([128, IC * DG], F16, tag="wv")
            wo_sb = pers.tile([128, 2 * D], F16, tag="wo")
            qt_sb = [pers.tile([128, S], F16, tag=f"qt{i}", name=f"qt{i}")
                     for i in range(2)]
            ktz_sb = [pers.tile([128, 2 * S], F16, tag=f"kt{i}", name=f"ktz{i}")
                      for i in range(2)]
            v_sb = pers.tile([128, KC * HPG * VW], F16, tag="v")
            ctxT_sb = [pers.tile([128, S], F16, tag=f"cx{i}", name=f"cx{i}")
                       for i in range(2)]
            bq_sb = pers.tile([128, 2], F32, tag="bq")
            bk_sb = pers.tile([128, 2], F32, tag="bk")
            bvb_sb = pers.tile([128, DG], F32, tag="bvb")
            recip_sb = pers.tile([1, 512], F16, tag="recip")

            # only what P1 needs up front; remaining weights stream in
            # behind the first x slices (keeps the first matmul early).
            nc.sync.dma_start(out=wk_sb[:], in_=wk[:])
            nc.sync.dma_start(out=bk_sb[:], in_=bk2.rearrange("c p -> p c"))
            for z in range(2):
                nc.vector.memset(ktz_sb[z][:], 0.0)

            # ---- deferred-emission queue (fills PE bubbles) ----
            deferred = []

            def flush(n=1 << 30):
                cnt = min(n, len(deferred))
                for fn in deferred[:cnt]:
                    fn()
                del deferred[:cnt]

            # ---- helper: project one 512-wide slice of x ----
            def qk_proj_slice(xdr, wsb3, sink, s4):
                xt = xin.tile([128, IC * 512], F16, tag="x", name=f"x_{s4}")
                nc.sync.dma_start(
                    out=xt.rearrange("p (k n) -> p k n", k=IC),
                    in_=xdr[:, :, s4 * 512:(s4 + 1) * 512].rearrange(
                        "k p n -> p k n"),
                )
                xt3 = xt.rearrange("p (k n) -> p k n", k=IC)
                for oc in range(2):
                    acc = ps.tile([128, 512], F32, tag="mm", name=f"acc_{s4}")
                    for ic in range(IC):
                        nc.tensor.matmul(
                            acc[:],
                            wsb3[:, ic, oc * 128:(oc + 1) * 128],
                            xt3[:, ic, :],
                            start=(ic == 0),
                            stop=(ic == IC - 1),
                        )
                    sink(oc, s4, acc)

            def qt_sink(oc, s4, acc):
                nc.vector.tensor_scalar_add(
                    qt_sb[oc][:, s4 * 512:(s4 + 1) * 512], acc[:],
                    bq_sb[:, oc:oc + 1],
                )

            def ktz_sink(oc, s4, acc):
                # head hsel of this pair -> col block (2*kb + hsel)*128, with
                # only d-rows 64*hsel..+64 populated (rest stays zero).
                z4 = ktz_sb[oc].rearrange("p (kb two m) -> p kb two m", two=2, m=128)
                a3 = acc.rearrange("p (kb m) -> p kb m", m=128)
                kb0 = 4 * s4
                for hsel in range(2):
                    nc.vector.tensor_scalar_add(
                        z4[64 * hsel:64 * hsel + 64, kb0:kb0 + 4, hsel, :],
                        a3[64 * hsel:64 * hsel + 64, :, :],
                        bk_sb[64 * hsel:64 * hsel + 64, oc:oc + 1],
                    )

            # ---- P1: k projection (q is projected inside the qs loop) ----
            wk3 = wk_sb.rearrange("p (k n) -> p k n", k=IC)
            wq3 = wq_sb.rearrange("p (k n) -> p k n", k=IC)
            for s4 in range(4):
                qk_proj_slice(xk, wk3, ktz_sink, s4)
                if s4 == 0:
                    # stream the remaining weights behind xk
                    nc.sync.dma_start(out=wv_sb[:], in_=wv[:])
                    nc.sync.dma_start(out=bvb_sb[:], in_=bvb[:])
                    nc.vector.memset(
                        v_sb.rearrange("p (k h e) -> p k h e",
                                       h=HPG, e=VW)[:, :, :, DH], 1.0)
                elif s4 == 1:
                    nc.sync.dma_start(out=wq_sb[:], in_=wq[:])
                    nc.sync.dma_start(out=bq_sb[:], in_=bq2.rearrange("c p -> p c"))
                elif s4 == 2:
                    nc.sync.dma_start(out=wo_sb[:], in_=wo[:])

            # ---- P3: v projection -> [S, 4*65] fp16 with ones columns ----
            wv3 = wv_sb.rearrange("p (k n) -> p k n", k=IC)
            v4 = v_sb.rearrange("p (k h e) -> p k h e", h=HPG, e=VW)
            for s4 in range(4):
                xt = xin.tile([128, IC * 512], F16, tag="x")
                nc.sync.dma_start(
                    out=xt.rearrange("p (k n) -> p k n", k=IC),
                    in_=xv[:, :, s4 * 512:(s4 + 1) * 512].rearrange("k p n -> p k n"),
                )
                xt3 = xt.rearrange("p (k n) -> p k n", k=IC)
                for j in range(4):  # key chunk kc = 4*s4 + j
                    kc = 4 * s4 + j
                    acc = ps.tile([128, 512], F32, tag="mm")
                    for ic in range(IC):
                        nc.tensor.matmul(
                            acc[:, 0:DG],
                            xt3[:, ic, j * 128:(j + 1) * 128],
                            wv3[:, ic, :],
                            start=(ic == 0),
                            stop=(ic == IC - 1),
                        )
                    nc.vector.tensor_add(
                        out=v4[:, kc, :, 0:DH],
                        in0=acc[:, 0:DG].rearrange("p (h e) -> p h e", e=DH),
                        in1=bvb_sb.rearrange("p (h e) -> p h e", e=DH),
                    )

            # ---- P4 + P5 per 512-wide query slice ----
            wo3 = wo_sb.rearrange("p (k n) -> p k n", k=2)

            def p5_tile(ib, oh):
                acc = ps.tile([128, 512], F32, tag="mm")
                for cc in range(2):
                    nc.tensor.matmul(
                        acc[:],
                        ctxT_sb[cc][:, ib * 128:(ib + 1) * 128],
                        wo3[:, cc, oh * 512:(oh + 1) * 512],
                        start=(cc == 0),
                        stop=(cc == 1),
                    )
                ot = outp.tile([128, 512], F16, tag="ot")
                nc.vector.tensor_copy(out=ot[:], in_=acc[:])
                nc.sync.dma_start(
                    out=out[ib * 128:(ib + 1) * 128, oh * 512:(oh + 1) * 512],
                    in_=ot[:],
                )

            for qs in range(4):  # 512-wide query slice
                qlo = qs * 512
                if qs == 0:
                    qk_proj_slice(xq, wq3, qt_sink, 0)
                for hg in range(HPG):
                    if hg == 3 and qs < 3:
                        # next slice's q projection: DMA prefetch + its DVE
                        # sink lands ahead of this head's normalize, so the
                        # next slice's first scores never wait on Vector.
                        qk_proj_slice(xq, wq3, qt_sink, qs + 1)
                    oc, ofs = hg // 2, 64 * (hg % 2)
                    hsel = hg % 2
                    ov = psov.tile([VW, 512], F32, tag="ov")
                    for kp in range(KC // 2):  # pairs of key blocks
                        sc = ps2.tile([128, 1024], F32, tag="sc")
                        for i in range(2):
                            kb = 2 * kp + i
                            nc.tensor.matmul(
                                sc[:, i * 512:(i + 1) * 512],
                                ktz_sb[oc][:, (2 * kb + hsel) * 128:
                                           (2 * kb + hsel + 1) * 128],
                                qt_sb[oc][:, qlo:qlo + 512],
                                start=True,
                                stop=True,
                            )
                        pt = ptp.tile([128, 1024], F16, tag="pt")
                        nc.scalar.activation(pt[:], sc[:], Exp, scale=0.125)
                        for i in range(2):
                            kb = 2 * kp + i
                            nc.tensor.matmul(
                                ov[:],
                                v_sb[:, (kb * HPG + hg) * VW:
                                     (kb * HPG + hg + 1) * VW],
                                pt[:, i * 512:(i + 1) * 512],
                                start=(kb == 0),
                                stop=(kb == KC - 1),
                            )
                        if kp in (3, 5, 7):
                            flush(1)  # a deferred P5 tile fills the PE bubble
                    # normalize: entirely off the Tensor engine
                    nc.vector.reciprocal(recip_sb[0:1, :], ov[DH:VW, :])
                    bcs = small.tile([DH, 512], F16, tag="bcs")
                    nc.gpsimd.partition_broadcast(bcs[:], recip_sb[0:1, :])
                    nc.vector.tensor_mul(
                        out=ctxT_sb[oc][ofs:ofs + DH, qlo:qlo + 512],
                        in0=ov[0:DH, :],
                        in1=bcs[:],
                    )

                # P5: partial output projection for the finished 512 rows,
                # deferred into the next slice's attention loop.
                for ibl in range(4):
                    ib = 4 * qs + ibl
                    for oh in range(2):
                        deferred.append(lambda ib=ib, oh=oh: p5_tile(ib, oh))
            flush()

    nc.compile()
    return nc


def _get_nc():
    if "nc" not in _NC_CACHE:
        _NC_CACHE["nc"] = _build_nc()
    return _NC_CACHE["nc"]


def _prep_inputs(Q, K, V, Wq, Wk, Wv, Wo, bq, bk, bv, bo):
    f = np.float32
    h = np.float16
    Q, K, V = (np.asarray(a, f) for a in (Q, K, V))
    Wq, Wk, Wv, Wo = (np.asarray(a, f) for a in (Wq, Wk, Wv, Wo))
    bq, bk, bv, bo = (np.asarray(a, f) for a in (bq, bk, bv, bo))

    xqs = [np.ascontiguousarray(Q[b].T).astype(h).reshape(IC, 128, S)
           for b in range(B)]
    xks = [np.ascontiguousarray(K[b].T).astype(h).reshape(IC, 128, S)
           for b in range(B)]
    xvs = [np.ascontiguousarray(V[b].T).astype(h).reshape(IC, 128, S)
           for b in range(B)]
    WqT, WkT, WvT, WoT = Wq.T, Wk.T, Wv.T, Wo.T

    def pack_w(wT_cols):
        # [1024, G] -> SBUF image [128 partitions, IC*G]: partition p holds
        # chunks [ic, p, :] contiguously
        G = wT_cols.shape[1]
        return np.ascontiguousarray(
            wT_cols.reshape(IC, 128, G).transpose(1, 0, 2).reshape(128, IC * G),
            dtype=h)

    def pack_wo(woT_rows):
        # [256, D] -> [128 partitions, 2*D]
        return np.ascontiguousarray(
            woT_rows.reshape(2, 128, D).transpose(1, 0, 2).reshape(128, 2 * D),
            dtype=h)

    in_maps = []
    for c in range(NCORES):
        b, g = c // 4, c % 4
        cols = slice(DG * g, DG * (g + 1))
        in_maps.append({
            "xq": xqs[b], "xk": xks[b], "xv": xvs[b],
            "wq": pack_w(WqT[:, cols]),
            "wk": pack_w(WkT[:, cols]),
            "wv": pack_w(WvT[:, cols]),
            "wo": pack_wo(WoT[cols, :]),
            "bq2": np.ascontiguousarray(bq[cols]).reshape(2, 128),
            "bk2": np.ascontiguousarray(bk[cols]).reshape(2, 128),
            "bvb": np.ascontiguousarray(np.broadcast_to(bv[cols], (128, DG))),
        })
    return in_maps, bo


def _assemble(results, bo):
    out = np.empty((B, S, D), np.float32)
    for b in range(B):
        acc = results[4 * b]["out"].astype(np.float32)
        for g in range(1, 4):
            acc += results[4 * b + g]["out"].astype(np.float32)
        out[b] = acc + bo
    return out


def kernel(**inputs):
    nc = _get_nc()
    in_maps, bo = _prep_inputs(**inputs)
    res = run_bass_kernel_spmd(nc, in_maps, core_ids=list(range(NCORES)))
    return _assemble(res.results, bo)


# revision 20
# speedup vs baseline: 1.1923x; 1.1105x over previous
"""MultiHeadAttention (B=2, S=2048, D=1024, H=16) on 8 Trainium2 NeuronCores.

Sharding: core c -> batch b = c // 4, head group g = c % 4 (4 of 16 heads =
256 of the 1024 projection columns). Cores are fully independent (no
collectives): each core writes its partial output projection (rows x D for
its 256 ctx columns) as fp16, and the host sums the 4 partials per batch
and adds bo. Per-core timeline:

  P1-P2: q/k projections for the core's 4 heads over the full sequence,
         produced directly transposed/head-major: qT,kT [256, S] fp16.
  P3:    v projection in natural layout [S, 4*65] fp16, with a ones column
         appended per head (yields softmax denominators for free in P4).
  P4:    per head and 1024-wide query slice: scoresT[k,q] = kT_blk.T @ qT
         (fp16 operands, fp32 PSUM) for two key blocks into one 2-bank
         PSUM tile, one 1024-wide exp on ScalarE with the 1/sqrt(64) scale
         folded in (scores are O(10): no max pass needed), then
         oT'[65,q] += v_blk.T @ pT. Row 64 of oT' is the softmax sum;
         normalize via DVE reciprocal + Pool partition_broadcast (keeps the
         normalize entirely off the Tensor engine) -> ctxT [256, S] fp16.
  P5:    partial output projection; each 128x512 tile is emitted deferred,
         interleaved into the next query slice's attention loop (from key
         pair 3 on, so the previous slice's last normalize has drained).

The next slice's q projection is emitted before the current slice's last
head so its Vector-engine sink isn't queued behind the slow reciprocal.
Matmul operands are fp16 (1 cycle/row on the PE); accumulation is fp32.
fp8/DoubleRow was evaluated and rejected: quantizing any attention operand
to fp8 puts 4-9% noise directly on the output (softmax averages signal and
noise alike), far over the 2e-2 budget.
"""

import numpy as np

import concourse.bacc as bacc
import concourse.mybir as mybir
from concourse.tile import TileContext
from concourse.bass_utils import run_bass_kernel_spmd

F32 = mybir.dt.float32
F16 = mybir.dt.float16

B, S, D = 2, 2048, 1024
H, DH = 16, 64
NCORES = 8
HPG = 4            # heads per core
DG = HPG * DH      # 256 projection cols per core
IC = D // 128      # 8 contraction chunks for the projections
KC = S // 128      # 16 key blocks
VW = DH + 1        # 65 = head dim + ones column

_NC_CACHE = {}


def _build_nc():
    nc = bacc.Bacc("TRN2", target_bir_lowering=False, num_devices=NCORES)

    xq = nc.dram_tensor("xq", [IC, 128, S], F16, kind="ExternalInput")
    xk = nc.dram_tensor("xk", [IC, 128, S], F16, kind="ExternalInput")
    xv = nc.dram_tensor("xv", [IC, 128, S], F16, kind="ExternalInput")
    # weights host-packed per-partition-contiguous: one 4KB descriptor per
    # partition on load
    wq = nc.dram_tensor("wq", [128, IC * DG], F16, kind="ExternalInput")
    wk = nc.dram_tensor("wk", [128, IC * DG], F16, kind="ExternalInput")
    wv = nc.dram_tensor("wv", [128, IC * DG], F16, kind="ExternalInput")
    wo = nc.dram_tensor("wo", [128, 2 * D], F16, kind="ExternalInput")
    bq2 = nc.dram_tensor("bq2", [2, 128], F32, kind="ExternalInput")
    bk2 = nc.dram_tensor("bk2", [2, 128], F32, kind="ExternalInput")
    bvb = nc.dram_tensor("bvb", [128, DG], F32, kind="ExternalInput")
    out = nc.dram_tensor("out", [S, D], F16, kind="ExternalOutput")

    Exp = mybir.ActivationFunctionType.Exp

    with nc.allow_low_precision(reason="fp16 attention internals"), \
            TileContext(nc) as tc:
        with (
            tc.tile_pool(name="persist", bufs=1) as pers,
            tc.tile_pool(name="xin", bufs=4) as xin,
            tc.tile_pool(name="pt", bufs=3) as ptp,
            tc.tile_pool(name="small", bufs=4) as small,
            tc.tile_pool(name="outp", bufs=3) as outp,
            tc.tile_pool(name="ps", bufs=2, space="PSUM") as ps,
            tc.tile_pool(name="ps2", bufs=2, space="PSUM") as ps2,
            tc.tile_pool(name="psov", bufs=2, space="PSUM") as psov,
        ):
            # ---- persistent SBUF ----
            wq_sb = pers.tile([128, IC * DG], F16, tag="wq")
            wk_sb = pers.tile([128, IC * DG], F16, tag="wk")
            wv_sb = pers.tile([128, IC * DG], F16, tag="wv")
            wo_sb = pers.tile([128, 2 * D], F16, tag="wo")
            qt_sb = [pers.tile([128, S], F16, tag=f"qt{i}", name=f"qt{i}")
                     for i in range(2)]
            ktz_sb = [pers.tile([128, 2 * S], F16, tag=f"kt{i}", name=f"ktz{i}")
                      for i in range(2)]
            v_sb = pers.tile([128, KC * HPG * VW], F16, tag="v")
            ctxT_sb = [pers.tile([128, S], F16, tag=f"cx{i}", name=f"cx{i}")
                       for i in range(2)]
            bq_sb = pers.tile([128, 2], F32, tag="bq")
            bk_sb = pers.tile([128, 2], F32, tag="bk")
            bvb_sb = pers.tile([128, DG], F32, tag="bvb")
            recip_sb = pers.tile([1, 512], F32, tag="recip")
            den_sb = pers.tile([1, 512], F32, tag="den")

            # only what P1 needs up front; remaining weights stream in
            # behind the first x slices (keeps the first matmul early).
            nc.sync.dma_start(out=wk_sb[:], in_=wk[:])
            nc.sync.dma_start(out=bk_sb[:], in_=bk2.rearrange("c p -> p c"))
            for z in range(2):
                nc.vector.memset(ktz_sb[z][:], 0.0)

            # ---- deferred-emission queue (fills PE bubbles) ----
            deferred = []

            def flush(n=1 << 30):
                cnt = min(n, len(deferred))
                for fn in deferred[:cnt]:
                    fn()
                del deferred[:cnt]

            # ---- helper: project one 512-wide slice of x ----
            def qk_proj_slice(xdr, wsb3, sink, s4):
                xt = xin.tile([128, IC * 512], F16, tag="x", name=f"x_{s4}")
                nc.sync.dma_start(
                    out=xt.rearrange("p (k n) -> p k n", k=IC),
                    in_=xdr[:, :, s4 * 512:(s4 + 1) * 512].rearrange(
                        "k p n -> p k n"),
                )
                xt3 = xt.rearrange("p (k n) -> p k n", k=IC)
                for oc in range(2):
                    acc = ps.tile([128, 512], F32, tag="mm", name=f"acc_{s4}")
                    for ic in range(IC):
                        nc.tensor.matmul(
                            acc[:],
                            wsb3[:, ic, oc * 128:(oc + 1) * 128],
                            xt3[:, ic, :],
                            start=(ic == 0),
                            stop=(ic == IC - 1),
                        )
                    sink(oc, s4, acc)

            def qt_sink(oc, s4, acc):
                nc.vector.tensor_scalar_add(
                    qt_sb[oc][:, s4 * 512:(s4 + 1) * 512], acc[:],
                    bq_sb[:, oc:oc + 1],
                )

            def ktz_sink(oc, s4, acc):
                # head hsel of this pair -> col block (2*kb + hsel)*128, with
                # only d-rows 64*hsel..+64 populated (rest stays zero).
                z4 = ktz_sb[oc].rearrange("p (kb two m) -> p kb two m", two=2, m=128)
                a3 = acc.rearrange("p (kb m) -> p kb m", m=128)
                kb0 = 4 * s4
                for hsel in range(2):
                    nc.vector.tensor_scalar_add(
                        z4[64 * hsel:64 * hsel + 64, kb0:kb0 + 4, hsel, :],
                        a3[64 * hsel:64 * hsel + 64, :, :],
                        bk_sb[64 * hsel:64 * hsel + 64, oc:oc + 1],
                    )

            # ---- P1: k projection (q is projected inside the qs loop) ----
            wk3 = wk_sb.rearrange("p (k n) -> p k n", k=IC)
            wq3 = wq_sb.rearrange("p (k n) -> p k n", k=IC)
            for s4 in range(4):
                qk_proj_slice(xk, wk3, ktz_sink, s4)
                if s4 == 0:
                    # stream the remaining weights behind xk
                    nc.sync.dma_start(out=wv_sb[:], in_=wv[:])
                    nc.sync.dma_start(out=bvb_sb[:], in_=bvb[:])
                    nc.vector.memset(
                        v_sb.rearrange("p (k h e) -> p k h e",
                                       h=HPG, e=VW)[:, :, :, DH], 1.0)
                elif s4 == 1:
                    nc.sync.dma_start(out=wq_sb[:], in_=wq[:])
                    nc.sync.dma_start(out=bq_sb[:], in_=bq2.rearrange("c p -> p c"))
                elif s4 == 2:
                    nc.sync.dma_start(out=wo_sb[:], in_=wo[:])

            # ---- P3: v projection -> [S, 4*65] fp16 with ones columns ----
            wv3 = wv_sb.rearrange("p (k n) -> p k n", k=IC)
            v4 = v_sb.rearrange("p (k h e) -> p k h e", h=HPG, e=VW)
            for s4 in range(4):
                xt = xin.tile([128, IC * 512], F16, tag="x")
                nc.sync.dma_start(
                    out=xt.rearrange("p (k n) -> p k n", k=IC),
                    in_=xv[:, :, s4 * 512:(s4 + 1) * 512].rearrange("k p n -> p k n"),
                )
                xt3 = xt.rearrange("p (k n) -> p k n", k=IC)
                for j in range(4):  # key chunk kc = 4*s4 + j
                    kc = 4 * s4 + j
                    acc = ps.tile([128, 512], F32, tag="mm")
                    for ic in range(IC):
                        nc.tensor.matmul(
                            acc[:, 0:DG],
                            xt3[:, ic, j * 128:(j + 1) * 128],
                            wv3[:, ic, :],
                            start=(ic == 0),
                            stop=(ic == IC - 1),
                        )
                    nc.vector.tensor_add(
                        out=v4[:, kc, :, 0:DH],
                        in0=acc[:, 0:DG].rearrange("p (h e) -> p h e", e=DH),
                        in1=bvb_sb.rearrange("p (h e) -> p h e", e=DH),
                    )

            # ---- P4 + P5 per 512-wide query slice ----
            wo3 = wo_sb.rearrange("p (k n) -> p k n", k=2)

            def p5_tile(ib, oh):
                acc = ps.tile([128, 512], F32, tag="mm")
                for cc in range(2):
                    nc.tensor.matmul(
                        acc[:],
                        ctxT_sb[cc][:, ib * 128:(ib + 1) * 128],
                        wo3[:, cc, oh * 512:(oh + 1) * 512],
                        start=(cc == 0),
                        stop=(cc == 1),
                    )
                ot = outp.tile([128, 512], F16, tag="ot")
                nc.vector.tensor_copy(out=ot[:], in_=acc[:])
                nc.sync.dma_start(
                    out=out[ib * 128:(ib + 1) * 128, oh * 512:(oh + 1) * 512],
                    in_=ot[:],
                )

            for qs in range(4):  # 512-wide query slice
                qlo = qs * 512
                if qs == 0:
                    qk_proj_slice(xq, wq3, qt_sink, 0)
                for hg in range(HPG):
                    if hg == 3 and qs < 3:
                        # next slice's q projection: DMA prefetch + its DVE
                        # sink lands ahead of this head's normalize, so the
                        # next slice's first scores never wait on Vector.
                        qk_proj_slice(xq, wq3, qt_sink, qs + 1)
                    oc, ofs = hg // 2, 64 * (hg % 2)
                    hsel = hg % 2
                    ov = psov.tile([VW, 512], F32, tag="ov")
                    for kp in range(KC // 2):  # pairs of key blocks
                        sc = ps2.tile([128, 1024], F32, tag="sc")
                        for i in range(2):
                            kb = 2 * kp + i
                            nc.tensor.matmul(
                                sc[:, i * 512:(i + 1) * 512],
                                ktz_sb[oc][:, (2 * kb + hsel) * 128:
                                           (2 * kb + hsel + 1) * 128],
                                qt_sb[oc][:, qlo:qlo + 512],
                                start=True,
                                stop=True,
                            )
                        pt = ptp.tile([128, 1024], F16, tag="pt")
                        nc.scalar.activation(pt[:], sc[:], Exp, scale=0.125)
                        for i in range(2):
                            kb = 2 * kp + i
                            nc.tensor.matmul(
                                ov[:],
                                v_sb[:, (kb * HPG + hg) * VW:
                                     (kb * HPG + hg + 1) * VW],
                                pt[:, i * 512:(i + 1) * 512],
                                start=(kb == 0),
                                stop=(kb == KC - 1),
                            )
                        # all deferred P5 tiles flush during head 0, before
                        # this slice's first ctx-mul is emitted (the tile-
                        # coarse ctxT dependency would otherwise bind them to
                        # the CURRENT slice's normalize)
                        if hg == 0:
                            flush(2 if kp >= 6 else (1 if kp >= 2 else 0))
                    # normalize: entirely off the Tensor engine. The
                    # denominator row is staged to SBUF so the fast
                    # approximate reciprocal (fp32-only, no PSUM) can be used.
                    nc.vector.tensor_copy(out=den_sb[:], in_=ov[DH:VW, :])
                    nc.vector.reciprocal_approx_fast(recip_sb[0:1, :],
                                                     den_sb[0:1, :])
                    bcs = small.tile([DH, 512], F32, tag="bcs")
                    nc.gpsimd.partition_broadcast(bcs[:], recip_sb[0:1, :])
                    nc.vector.tensor_mul(
                        out=ctxT_sb[oc][ofs:ofs + DH, qlo:qlo + 512],
                        in0=ov[0:DH, :],
                        in1=bcs[:],
                    )

                # P5: partial output projection for the finished 512 rows,
                # deferred into the next slice's attention loop.
                for ibl in range(4):
                    ib = 4 * qs + ibl
                    for oh in range(2):
                        deferred.append(lambda ib=ib, oh=oh: p5_tile(ib, oh))
            flush()

    nc.compile()
    return nc


def _get_nc():
    if "nc" not in _NC_CACHE:
        _NC_CACHE["nc"] = _build_nc()
    return _NC_CACHE["nc"]


def _prep_inputs(Q, K, V, Wq, Wk, Wv, Wo, bq, bk, bv, bo):
    f = np.float32
    h = np.float16
    Q, K, V = (np.asarray(a, f) for a in (Q, K, V))
    Wq, Wk, Wv, Wo = (np.asarray(a, f) for a in (Wq, Wk, Wv, Wo))
    bq, bk, bv, bo = (np.asarray(a, f) for a in (bq, bk, bv, bo))

    xqs = [np.ascontiguousarray(Q[b].T).astype(h).reshape(IC, 128, S)
           for b in range(B)]
    xks = [np.ascontiguousarray(K[b].T).astype(h).reshape(IC, 128, S)
           for b in range(B)]
    xvs = [np.ascontiguousarray(V[b].T).astype(h).reshape(IC, 128, S)
           for b in range(B)]
    WqT, WkT, WvT, WoT = Wq.T, Wk.T, Wv.T, Wo.T

    def pack_w(wT_cols):
        # [1024, G] -> SBUF image [128 partitions, IC*G]: partition p holds
        # chunks [ic, p, :] contiguously
        G = wT_cols.shape[1]
        return np.ascontiguousarray(
            wT_cols.reshape(IC, 128, G).transpose(1, 0, 2).reshape(128, IC * G),
            dtype=h)

    def pack_wo(woT_rows):
        # [256, D] -> [128 partitions, 2*D]
        return np.ascontiguousarray(
            woT_rows.reshape(2, 128, D).transpose(1, 0, 2).reshape(128, 2 * D),
            dtype=h)

    in_maps = []
    for c in range(NCORES):
        b, g = c // 4, c % 4
        cols = slice(DG * g, DG * (g + 1))
        in_maps.append({
            "xq": xqs[b], "xk": xks[b], "xv": xvs[b],
            "wq": pack_w(WqT[:, cols]),
            "wk": pack_w(WkT[:, cols]),
            "wv": pack_w(WvT[:, cols]),
            "wo": pack_wo(WoT[cols, :]),
            "bq2": np.ascontiguousarray(bq[cols]).reshape(2, 128),
            "bk2": np.ascontiguousarray(bk[cols]).reshape(2, 128),
            "bvb": np.ascontiguousarray(np.broadcast_to(bv[cols], (128, DG))),
        })
    return in_maps, bo


def _assemble(results, bo):
    out = np.empty((B, S, D), np.float32)
    for b in range(B):
        acc = results[4 * b]["out"].astype(np.float32)
        for g in range(1, 4):
            acc += results[4 * b + g]["out"].astype(np.float32)
        out[b] = acc + bo
    return out


def kernel(**inputs):
    nc = _get_nc()
    in_maps, bo = _prep_inputs(**inputs)
    res = run_bass_kernel_spmd(nc, in_maps, core_ids=list(range(NCORES)))
    return _assemble(res.results, bo)
